# revision 10
# baseline (speedup 1.0000x reference)
"""Trainium2 Bass kernel for 2-layer bipartite GATv2 (users <-> items).

Strategy (8 NeuronCores):
  * Edges are assigned to cores by their USER node (u // USH); each core owns a
    contiguous shard of USH=12544 users.
  * u->i phase (rel 0): edge-parallel. Gather source features from the LOCAL
    xl_u table (dma_gather, int16 idx), scatter per-edge messages into a
    full-item accumulator [20480, 130] via one-hot matmuls in PSUM, then one
    AllReduce(add) combines partial item sums across cores.
  * i->u phase (rel 1): destination-sharded; gathers from the full item table;
    no collective.
  * Destination-side features (xr) are NEVER gathered: per destination block
    the xr rows live in SBUF ([128, nblocks*128] bf16, written by the dense
    phase), and each edge tile receives xr[dst] via a one-hot broadcast matmul
      psX  = P_t^T.T @ xr_blk     (P^T built by DVE is_equal against a
                                   partition-replicating DMA of ldst rows)
      psX += I.T @ XS             (PE-fused add => T = XS + XR, fp32 PSUM)
      LR   = Prelu_0.2(psX)       (ACT reads PSUM; exact leaky-relu)
  * Remaining per-edge math runs as slab (G=8 tiles) DVE ops in bf16 2x mode
    with broadcast APs -- no per-partition scalar-ptr ops (those cost ~1.1us).
      logit = reduce_X(LR * a_bcast); e = exp(logit) -> mm[:,:,128:130]
      mm[:,:,h*64:(h+1)*64] = XS_h * e_h;  P[e,t,d] = (ldst == iota)
      psum[block] += P_t.T @ mm_t          (scatter + denominators)
  * Softmax max-subtraction is skipped (logits are O(0.1) at this init scale);
    all-zero bias vectors skip their adds.
"""
import sys

for _p in ("/opt/trn_rl_repo", "/opt/pypackages"):
    if _p not in sys.path:
        sys.path.insert(0, _p)

import numpy as np
import ml_dtypes

import concourse.bacc as bacc
import concourse.bass as bass
import concourse.tile as tile
from concourse import mybir, library_config
from concourse.masks import make_identity

BF16 = ml_dtypes.bfloat16
F32 = np.float32
ALPHA = 0.2  # leaky relu slope
EPS = 1e-16
LRELU_ON_ACT = True  # HW: ACT Prelu(alpha); False: DVE max(0.2x, x) (CoreSim path)

FULL_CFG = dict(NU=100000, NI=20000, E=250000, NC=8, USH=12544, NIP=20480, G=8)


# ----------------------------------------------------------------------------
# host-side planning
# ----------------------------------------------------------------------------

def _wrap_idxs(idx: np.ndarray) -> np.ndarray:
    """dma_gather int16 index layout: [128, n/16], j -> [j%16, j//16], x8 replicas."""
    n = idx.shape[0]
    assert n % 16 == 0
    a = np.empty((16, n // 16), np.int16)
    a[np.arange(n) % 16, np.arange(n) // 16] = idx.astype(np.int16)
    return np.tile(a, (8, 1))


def _slot_fill(key, Ks, vals_dummies):
    """Place per-edge values into padded per-block tile slots."""
    nslot = int(Ks.sum()) * 128
    offs = np.zeros(len(Ks) + 1, np.int64)
    offs[1:] = np.cumsum(Ks * 128)
    order = np.argsort(key, kind="stable")
    sk = key[order]
    block_start = np.searchsorted(sk, np.arange(len(Ks)))
    rank = np.arange(len(sk)) - block_start[sk]
    pos = offs[sk] + rank
    outs = []
    for vals, dummy, dt in vals_dummies:
        a = np.full(nslot, dummy, dt)
        a[pos] = vals[order].astype(dt)
        outs.append(a)
    return outs


def plan(edge_u, edge_i, cfg):
    """Build the global tile schedules (KA, KB) and per-core edge arrays."""
    NC, USH, NIP = cfg["NC"], cfg["USH"], cfg["NIP"]
    NBI, NBU = NIP // 128, USH // 128
    owner = edge_u // USH
    per_core = []
    cntA = np.zeros((NC, NBI), np.int64)
    cntB = np.zeros((NC, NBU), np.int64)
    for c in range(NC):
        m = owner == c
        eu = edge_u[m] - c * USH
        ei = edge_i[m]
        cntA[c] = np.bincount(ei // 128, minlength=NBI)
        cntB[c] = np.bincount(eu // 128, minlength=NBU)
        per_core.append((eu, ei))
    KA = np.maximum(1, -(-cntA.max(0) // 128)).astype(np.int64)
    KB = np.maximum(1, -(-cntB.max(0) // 128)).astype(np.int64)

    cores = []
    for c in range(NC):
        eu, ei = per_core[c]
        srcA, ldA = _slot_fill(ei // 128, KA, [
            (eu, 0, np.int16), (ei % 128, 200, np.int16)])
        srcB, ldB = _slot_fill(eu // 128, KB, [
            (ei, 0, np.int16), (eu % 128, 200, np.int16)])
        NTA, NTB = int(KA.sum()), int(KB.sum())
        cores.append(dict(
            srcA=_wrap_idxs(srcA),
            ldA=np.ascontiguousarray(ldA.reshape(NTA, 128).T.astype(BF16)),
            ldRA=np.ascontiguousarray(ldA.reshape(NTA, 128).astype(BF16)),
            srcB=_wrap_idxs(srcB),
            ldB=np.ascontiguousarray(ldB.reshape(NTB, 128).T.astype(BF16)),
            ldRB=np.ascontiguousarray(ldB.reshape(NTB, 128).astype(BF16)),
        ))
    return KA, KB, cores


# ----------------------------------------------------------------------------
# kernel builder
# ----------------------------------------------------------------------------

def build(cfg, KA, KB, nz):
    NC, USH, NIP, G = cfg["NC"], cfg["USH"], cfg["NIP"], cfg["G"]
    NBI, NBU = NIP // 128, USH // 128
    NTA, NTB = int(KA.sum()), int(KB.sum())
    blkA = np.repeat(np.arange(NBI), KA)
    blkB = np.repeat(np.arange(NBU), KB)
    bf = mybir.dt.bfloat16
    f32 = mybir.dt.float32
    i16 = mybir.dt.int16

    nc = bacc.Bacc("TRN2", num_devices=NC, num_swdge_queues=4)

    xuT = nc.dram_tensor("xuT", [64, USH], bf, kind="ExternalInput")
    xiT = nc.dram_tensor("xiT", [128, NIP], bf, kind="ExternalInput")
    wpu = nc.dram_tensor("wpu", [64, 128], bf, kind="ExternalInput")
    wpi = nc.dram_tensor("wpi", [128, 128], bf, kind="ExternalInput")
    bpu = nc.dram_tensor("bpu", [128, 1], f32, kind="ExternalInput")
    bpi = nc.dram_tensor("bpi", [128, 1], f32, kind="ExternalInput")
    iota_in = nc.dram_tensor("iota", [128, 128], bf, kind="ExternalInput")
    iotc_in = nc.dram_tensor("iotc", [128, 1], bf, kind="ExternalInput")
    idnb_in = nc.dram_tensor("idnb", [128, 128], bf, kind="ExternalInput")
    w_in, bb_in, ab_in, ob_in = {}, {}, {}, {}
    for l in range(2):
        for r in range(2):
            w_in[("l", l, r)] = nc.dram_tensor(f"wl{l}{r}", [128, 128], bf, kind="ExternalInput")
            w_in[("r", l, r)] = nc.dram_tensor(f"wr{l}{r}", [128, 128], bf, kind="ExternalInput")
            bb_in[("l", l, r)] = nc.dram_tensor(f"blb{l}{r}", [128, 128], f32, kind="ExternalInput")
            bb_in[("r", l, r)] = nc.dram_tensor(f"brb{l}{r}", [128, 128], f32, kind="ExternalInput")
            ab_in[(l, r)] = nc.dram_tensor(f"ab{l}{r}", [128, 128], bf, kind="ExternalInput")
            ob_in[(l, r)] = nc.dram_tensor(f"ob{l}{r}", [128, 128], f32, kind="ExternalInput")
    srcA_in = nc.dram_tensor("srcA", [128, NTA * 8], i16, kind="ExternalInput")
    ldA_in = nc.dram_tensor("ldA", [128, NTA], bf, kind="ExternalInput")
    ldRA_in = nc.dram_tensor("ldRA", [NTA, 128], bf, kind="ExternalInput")
    srcB_in = nc.dram_tensor("srcB", [128, NTB * 8], i16, kind="ExternalInput")
    ldB_in = nc.dram_tensor("ldB", [128, NTB], bf, kind="ExternalInput")
    ldRB_in = nc.dram_tensor("ldRB", [NTB, 128], bf, kind="ExternalInput")
    zu_out = nc.dram_tensor("zu_out", [USH, 128], f32, kind="ExternalOutput")
    zi_out = nc.dram_tensor("zi_out", [NIP, 128], f32, kind="ExternalOutput")

    AluOp = mybir.AluOpType
    ActFn = mybir.ActivationFunctionType
    qn = [0]

    with tile.TileContext(nc, num_cores=NC) as tc:
        with (
            tc.tile_pool(name="const", bufs=1) as const,
            tc.tile_pool(name="sbuf", bufs=2) as sbuf,
            tc.tile_pool(name="psum", bufs=2, space="PSUM") as psum,
            tc.tile_pool(name="dram", bufs=1, space="DRAM") as dram,
        ):
            nc.gpsimd.load_library(library_config.mlp)

            def cload(src, shape, dtype, name):
                t = const.tile(shape, dtype, name=name, tag=name)
                nc.sync.dma_start(t[:], src[:])
                return t

            iota_t = cload(iota_in, [128, 128], bf, "iota_t")
            iotc_t = cload(iotc_in, [128, 1], bf, "iotc_t")
            idnb_t = cload(idnb_in, [128, 128], bf, "idnb_t")
            wpu_t = cload(wpu, [64, 128], bf, "wpu_t")
            wpi_t = cload(wpi, [128, 128], bf, "wpi_t")
            bpu_t = cload(bpu, [128, 1], f32, "bpu_t")
            bpi_t = cload(bpi, [128, 1], f32, "bpi_t")
            w_t, bb_t, ab_t, ob_t = {}, {}, {}, {}
            for l in range(2):
                for r in range(2):
                    for s in ("l", "r"):
                        w_t[(s, l, r)] = cload(w_in[(s, l, r)], [128, 128], bf, f"w{s}{l}{r}_t")
                        if nz["tab"]:
                            bb_t[(s, l, r)] = cload(bb_in[(s, l, r)], [128, 128], f32, f"b{s}b{l}{r}_t")
                    ab_t[(l, r)] = cload(ab_in[(l, r)], [128, 128], bf, f"ab{l}{r}_t")
                    if nz["out"]:
                        ob_t[(l, r)] = cload(ob_in[(l, r)], [128, 128], f32, f"ob{l}{r}_t")
            srcA_t = cload(srcA_in, [128, NTA * 8], i16, "srcA_t")
            ldA_t = cload(ldA_in, [128, NTA], bf, "ldA_t")
            srcB_t = cload(srcB_in, [128, NTB * 8], i16, "srcB_t")
            ldB_t = cload(ldB_in, [128, NTB], bf, "ldB_t")
            ident_t = const.tile([128, 128], f32, name="ident_t", tag="ident_t")
            make_identity(nc, ident_t[:])

            def dtile(name, shape, dtype, shared=False):
                return dram.tile(shape, dtype, name=name, tag=name,
                                 addr_space="Shared" if shared else "Local")

            zuT = [dtile(f"zuT{i}", [128, USH], bf) for i in range(2)]
            ziT = [dtile(f"ziT{i}", [128, NIP], bf) for i in range(2)]
            tbl = {}
            for l in range(2):
                tbl[("xlu", l)] = dtile(f"xlu{l}", [USH, 128], bf)
                tbl[("xli", l)] = dtile(f"xli{l}", [NIP, 128], bf)
            accA = [dtile(f"accA{l}", [NIP, 130], f32) for l in range(2)]
            accAr = [dtile(f"accAr{l}", [NIP, 130], f32, shared=True) for l in range(2)]

            # ---- initial projections (feature-major out) --------------------
            def init_proj(xT, w_tile, b_col, dstT, ncols, kdim):
                for c0 in range(0, ncols, 512):
                    w = min(512, ncols - c0)
                    xc = sbuf.tile([128, 512], bf, name="xc_init", tag="xcI")
                    nc.sync.dma_start(xc[:kdim, :w], xT[:kdim, c0:c0 + w])
                    ps = psum.tile([128, 512], f32, name="ps_init", tag="psD")
                    nc.tensor.matmul(ps[:, :w], lhsT=w_tile[:kdim, :], rhs=xc[:kdim, :w],
                                     start=True, stop=True)
                    stg = sbuf.tile([128, 512], bf, name="stg_init", tag="stgD")
                    if nz["bp"]:
                        nc.scalar.activation(stg[:, :w], ps[:, :w], ActFn.Identity,
                                             bias=b_col[:])
                    else:
                        nc.scalar.copy(stg[:, :w], ps[:, :w])
                    nc.sync.dma_start(dstT[:, c0:c0 + w], stg[:, :w])

            init_proj(xuT, wpu_t, bpu_t, zuT[0], USH, 64)
            init_proj(xiT, wpi_t, bpi_t, ziT[0], NIP, 128)

            # ---- dense tables: xl -> HBM (gather source), xr -> SBUF -------
            def make_tables(zT, nblocks, wL, bL, dstL_hbm, wR, bR, dstR_sb):
                for n0 in range(0, nblocks, 4):
                    nb = min(4, nblocks - n0)
                    zt = sbuf.tile([128, 4 * 128], bf, name="zt_d", tag="ztD")
                    nc.sync.dma_start(zt[:, :nb * 128], zT[:, n0 * 128:(n0 + nb) * 128])
                    stg = sbuf.tile([128, 4, 128], bf, name="stg_tab", tag="stgT")
                    for k in range(nb):
                        psl = psum.tile([128, 512], f32, name="ps_tabl", tag="psD")
                        nc.tensor.matmul(psl[:, :128], lhsT=zt[:, k * 128:(k + 1) * 128],
                                         rhs=wL[:], start=True, stop=True)
                        if bL is not None:
                            nc.vector.tensor_tensor(out=stg[:, k, :], in0=psl[:, :128],
                                                    in1=bL[:], op=AluOp.add)
                        else:
                            nc.scalar.copy(stg[:, k, :], psl[:, :128])
                        psr = psum.tile([128, 128], f32, name="ps_tabr", tag="psX",
                                        bufs=3)
                        nc.tensor.matmul(psr[:], lhsT=zt[:, k * 128:(k + 1) * 128],
                                         rhs=wR[:], start=True, stop=True)
                        n = n0 + k
                        if bR is not None:
                            nc.vector.tensor_tensor(out=dstR_sb[:, n * 128:(n + 1) * 128],
                                                    in0=psr[:], in1=bR[:], op=AluOp.add)
                        else:
                            nc.scalar.copy(dstR_sb[:, n * 128:(n + 1) * 128], psr[:])
                    nc.sync.dma_start(
                        dstL_hbm[n0 * 128:(n0 + nb) * 128, :].rearrange(
                            "(t p) f -> p t f", p=128),
                        stg[:, :nb, :])

            # ---- edge phase -------------------------------------------------
            def edge_phase(tile_blk, Ks, src_tbl, xr_sb, src_idx, ld_t, ldR_in,
                           avec, flush):
                NT = len(tile_blk)
                n_in_blk = 0
                ps = None
                iota_b = iota_t[:].rearrange("p (g f) -> p g f", g=1)
                iotc_b = iotc_t[:].rearrange("p (g f) -> p g f", g=1)
                avec_b = avec[:].rearrange("p (g f) -> p g f", g=1)
                for t0 in range(0, NT, G):
                    g = min(G, NT - t0)
                    ni = g * 128
                    xs = sbuf.tile([128, G, 128], bf, name="xs_e", tag="xsE", bufs=3)
                    nc.gpsimd.dma_gather(xs[:, :g, :], src_tbl[:],
                                         src_idx[:, t0 * 8:(t0 + g) * 8], ni, ni, 128,
                                         queue_num=qn[0] % 4)
                    qn[0] += 1
                    # replicate ldst rows down partitions: ldrep[p, k, d] = ldst[t0+k, d]
                    ldrep = sbuf.tile([128, G, 128], bf, name="ldrep_e", tag="ldrepE",
                                      bufs=2)
                    nc.sync.dma_start(
                        ldrep[:, :g, :],
                        ldR_in[t0:t0 + g, :].rearrange("t d -> (t d)")
                        .rearrange("(g x) -> g x", g=1)
                        .to_broadcast([128, g * 128])
                        .rearrange("p (t d) -> p t d", t=g))
                    ppt = sbuf.tile([128, G, 128], bf, name="ppt_e", tag="pptE", bufs=3)
                    nc.vector.tensor_tensor(
                        out=ppt[:, :g, :],
                        in0=iotc_b.to_broadcast([128, g, 128]),
                        in1=ldrep[:, :g, :], op=AluOp.is_equal)
                    lr = sbuf.tile([128, G, 128], bf, name="lr_e", tag="lrE", bufs=3)
                    for k in range(g):
                        b = tile_blk[t0 + k]
                        psx = psum.tile([128, 128], f32, name="ps_x", tag="psX", bufs=3)
                        nc.tensor.matmul(psx[:], lhsT=ppt[:, k, :],
                                         rhs=xr_sb[:, b * 128:(b + 1) * 128],
                                         start=True, stop=False)
                        nc.tensor.matmul(psx[:], lhsT=idnb_t[:], rhs=xs[:, k, :],
                                         start=False, stop=True)
                        if LRELU_ON_ACT:
                            nc.scalar.activation(lr[:, k, :], psx[:], ActFn.Prelu,
                                                 alpha=ALPHA)
                        else:
                            nc.vector.scalar_tensor_tensor(
                                out=lr[:, k, :], in0=psx[:], scalar=ALPHA,
                                in1=psx[:], op0=AluOp.mult, op1=AluOp.max)
                    uu = sbuf.tile([128, G, 128], bf, name="uu_e", tag="uuE", bufs=3)
                    nc.vector.tensor_tensor(
                        out=uu[:, :g, :], in0=lr[:, :g, :],
                        in1=avec_b.to_broadcast([128, g, 128]), op=AluOp.mult)
                    lg = sbuf.tile([128, G, 2], f32, name="lg_e", tag="lgE", bufs=3)
                    nc.vector.tensor_reduce(
                        out=lg[:, :g, :],
                        in_=uu[:, :g, :].rearrange("p g (h d) -> p g h d", h=2),
                        axis=mybir.AxisListType.X, op=AluOp.add)
                    mm = sbuf.tile([128, G, 130], bf, name="mm_e", tag="mmE", bufs=3)
                    nc.scalar.activation(mm[:, :g, 128:130], lg[:, :g, :], ActFn.Exp)
                    for h in range(2):
                        nc.vector.tensor_tensor(
                            out=mm[:, :g, h * 64:(h + 1) * 64],
                            in0=xs[:, :g, h * 64:(h + 1) * 64],
                            in1=mm[:, :g, 128 + h:129 + h].to_broadcast([128, g, 64]),
                            op=AluOp.mult)
                    pp = sbuf.tile([128, G, 128], bf, name="pp_e", tag="ppE", bufs=3)
                    nc.vector.tensor_tensor(
                        out=pp[:, :g, :],
                        in0=iota_b.to_broadcast([128, g, 128]),
                        in1=ld_t[:, t0:t0 + g].to_broadcast([128, g, 128]),
                        op=AluOp.is_equal)
                    for k in range(g):
                        t = t0 + k
                        b = tile_blk[t]
                        if n_in_blk == 0:
                            ps = psum.tile([128, 130], f32, name="ps_e", tag="psE",
                                           bufs=3)
                        first = n_in_blk == 0
                        last = n_in_blk == Ks[b] - 1
                        nc.tensor.matmul(ps[:, 0:130], lhsT=pp[:, k, :], rhs=mm[:, k, :],
                                         start=first, stop=last)
                        if last:
                            flush(b, ps)
                            n_in_blk = 0
                        else:
                            n_in_blk += 1

            # ---- normalize --------------------------------------------------
            def normalize(acc3, nb, obias, do_relu, out3):
                dpe = sbuf.tile([128, 4, 2], f32, name="dpe_n", tag="dpeN")
                nc.vector.tensor_scalar_add(dpe[:, :nb, :], acc3[:, :, 128:130], EPS)
                rcp = sbuf.tile([128, 4, 2], f32, name="rcp_n", tag="rcpN")
                nc.vector.reciprocal(rcp[:, :nb, :], dpe[:, :nb, :])
                for h in range(2):
                    nc.vector.tensor_tensor(
                        out=out3[:, :, h * 64:(h + 1) * 64],
                        in0=acc3[:, :, h * 64:(h + 1) * 64],
                        in1=rcp[:, :nb, h:h + 1].to_broadcast([128, nb, 64]),
                        op=AluOp.mult)
                if obias is not None:
                    nc.vector.tensor_tensor(
                        out=out3[:], in0=out3[:],
                        in1=obias[:].rearrange("p (g f) -> p g f", g=1)
                        .to_broadcast([128, nb, 128]),
                        op=AluOp.add)
                if do_relu:
                    nc.vector.tensor_scalar_max(out3[:], out3[:], 0.0)

            # ---- layers -----------------------------------------------------
            for l in range(2):
                bbt = (lambda s, r: bb_t[(s, l, r)]) if nz["tab"] else (lambda s, r: None)
                obt = (lambda r: ob_t[(l, r)]) if nz["out"] else (lambda r: None)
                # xr tables, SBUF resident for the one-hot broadcast matmuls
                xru_sb = sbuf.tile([128, NBU * 128], bf, name="xru_sb", tag="xruSB")
                xri_sb = sbuf.tile([128, NBI * 128], bf, name="xri_sb", tag="xriSB")
                make_tables(zuT[l], NBU,
                            w_t[("l", l, 0)], bbt("l", 0), tbl[("xlu", l)],
                            w_t[("r", l, 1)], bbt("r", 1), xru_sb)
                make_tables(ziT[l], NBI,
                            w_t[("l", l, 1)], bbt("l", 1), tbl[("xli", l)],
                            w_t[("r", l, 0)], bbt("r", 0), xri_sb)

                # ---- A phase: u -> i (edge parallel + AllReduce) ----------
                def flushA(b, ps, l=l):
                    stg = sbuf.tile([128, 130], f32, name="stg_fa", tag="stgFA")
                    nc.scalar.copy(stg[:], ps[:])
                    nc.sync.dma_start(accA[l][b * 128:(b + 1) * 128, :], stg[:])

                edge_phase(blkA, KA, tbl[("xlu", l)], xri_sb,
                           srcA_t, ldA_t, ldRA_in, ab_t[(l, 0)], flushA)
                nc.gpsimd.collective_compute(
                    "AllReduce", AluOp.add,
                    replica_groups=[list(range(NC))],
                    ins=[accA[l][:].opt()], outs=[accAr[l][:].opt()])

                # ---- post A: zi_new ---------------------------------------
                for b0 in range(0, NBI, 4):
                    nb = min(4, NBI - b0)
                    acc = sbuf.tile([128, 4, 130], f32, name="acc_pa", tag="accPA")
                    nc.sync.dma_start(
                        acc[:, :nb, :],
                        accAr[l][b0 * 128:(b0 + nb) * 128, :].rearrange(
                            "(t p) c -> p t c", p=128))
                    zi_new = sbuf.tile([128, 4, 128], f32, name="zin_pa", tag="zinPA")
                    normalize(acc[:, :nb, :], nb, obt(0), l == 0, zi_new[:, :nb, :])
                    if l == 0:
                        stg = sbuf.tile([128, 4 * 128], bf, name="stg_tp", tag="stgTP")
                        for k in range(nb):
                            pst = psum.tile([128, 128], f32, name="ps_tp", tag="psX",
                                            bufs=3)
                            nc.tensor.transpose(pst[:], zi_new[:, k, :], ident_t[:])
                            nc.scalar.copy(stg[:, k * 128:(k + 1) * 128], pst[:])
                        nc.sync.dma_start(ziT[1][:, b0 * 128:(b0 + nb) * 128],
                                          stg[:, :nb * 128])
                    else:
                        nc.sync.dma_start(
                            zi_out[b0 * 128:(b0 + nb) * 128, :].rearrange(
                                "(t p) f -> p t f", p=128),
                            zi_new[:, :nb, :])

                # ---- B phase: i -> u (dst sharded, local) -----------------
                def flushB(j, ps, l=l):
                    zu_new = sbuf.tile([128, 1, 128], f32, name="zun_fb", tag="zunFB")
                    ps3 = ps[:].rearrange("p (g c) -> p g c", g=1)
                    normalize(ps3, 1, obt(1), l == 0, zu_new[:, :1, :])
                    if l == 0:
                        pst = psum.tile([128, 128], f32, name="ps_tpb", tag="psX",
                                        bufs=3)
                        nc.tensor.transpose(pst[:], zu_new[:, 0, :], ident_t[:])
                        stg = sbuf.tile([128, 128], bf, name="stg_tpb", tag="stgTPB")
                        nc.scalar.copy(stg[:], pst[:])
                        nc.sync.dma_start(zuT[1][:, j * 128:(j + 1) * 128], stg[:])
                    else:
                        nc.sync.dma_start(zu_out[j * 128:(j + 1) * 128, :],
                                          zu_new[:, 0, :])

                edge_phase(blkB, KB, tbl[("xli", l)], xru_sb,
                           srcB_t, ldB_t, ldRB_in, ab_t[(l, 1)], flushB)

    nc.compile()
    return nc


# ----------------------------------------------------------------------------
# host wrapper
# ----------------------------------------------------------------------------

def _bcast_row(v, dtype):
    return np.ascontiguousarray(
        np.tile(np.asarray(v, F32).reshape(1, -1), (128, 1)).astype(dtype))


def prep_in_maps(inputs, cfg, cores):
    NC, USH, NIP = cfg["NC"], cfg["USH"], cfg["NIP"]
    NU, NI = cfg["NU"], cfg["NI"]
    x_user = np.asarray(inputs["x_user"], F32)
    x_item = np.asarray(inputs["x_item"], F32)
    xu_pad = np.zeros((NC * USH, 64), F32)
    xu_pad[:NU] = x_user
    xi_pad = np.zeros((NIP, 128), F32)
    xi_pad[:NI] = x_item
    xiT = np.ascontiguousarray(xi_pad.T.astype(BF16))

    Wl, bl = np.asarray(inputs["Wl"], F32), np.asarray(inputs["bl"], F32)
    Wr, br = np.asarray(inputs["Wr"], F32), np.asarray(inputs["br"], F32)
    att, obias = np.asarray(inputs["att"], F32), np.asarray(inputs["bias"], F32)

    shared = {
        "xiT": xiT,
        "wpu": np.asarray(inputs["Wp_user"], F32).astype(BF16),
        "wpi": np.asarray(inputs["Wp_item"], F32).astype(BF16),
        "bpu": np.asarray(inputs["bp_user"], F32).reshape(128, 1),
        "bpi": np.asarray(inputs["bp_item"], F32).reshape(128, 1),
        "iota": _bcast_row(np.arange(128), BF16),
        "iotc": np.arange(128, dtype=F32).reshape(128, 1).astype(BF16),
        "idnb": np.eye(128, dtype=F32).astype(BF16),
    }
    for l in range(2):
        for r in range(2):
            shared[f"wl{l}{r}"] = Wl[l, r].astype(BF16)
            shared[f"wr{l}{r}"] = Wr[l, r].astype(BF16)
            shared[f"blb{l}{r}"] = _bcast_row(bl[l, r], F32)
            shared[f"brb{l}{r}"] = _bcast_row(br[l, r], F32)
            shared[f"ab{l}{r}"] = _bcast_row(att[l, r].reshape(128), BF16)
            shared[f"ob{l}{r}"] = _bcast_row(obias[l, r], F32)

    in_maps = []
    for c in range(NC):
        m = dict(shared)
        m["xuT"] = np.ascontiguousarray(
            xu_pad[c * USH:(c + 1) * USH].T.astype(BF16))
        m.update(cores[c])
        in_maps.append(m)
    return in_maps


def bias_flags(inputs):
    return dict(
        bp=bool(np.any(np.asarray(inputs["bp_user"])) or np.any(np.asarray(inputs["bp_item"]))),
        tab=bool(np.any(np.asarray(inputs["bl"])) or np.any(np.asarray(inputs["br"]))),
        out=bool(np.any(np.asarray(inputs["bias"]))),
    )


_BUILT = {}
LAST_RESULTS = None


def kernel(x_user, x_item, Wp_user, bp_user, Wp_item, bp_item,
           Wl, bl, Wr, br, att, bias, edge_src, edge_dst,
           trace=False):
    global LAST_RESULTS
    from concourse.bass_utils import run_bass_kernel_spmd

    cfg = FULL_CFG
    inputs = dict(x_user=x_user, x_item=x_item, Wp_user=Wp_user,
                  bp_user=bp_user, Wp_item=Wp_item, bp_item=bp_item,
                  Wl=Wl, bl=bl, Wr=Wr, br=br, att=att, bias=bias)
    eu = np.asarray(edge_src, np.int64)
    ei = np.asarray(edge_dst, np.int64)
    KA, KB, cores = plan(eu, ei, cfg)
    nz = bias_flags(inputs)

    key = (tuple(KA), tuple(KB), tuple(sorted(nz.items())), LRELU_ON_ACT)
    if key not in _BUILT:
        _BUILT.clear()
        _BUILT[key] = build(cfg, KA, KB, nz)
    nc = _BUILT[key]

    in_maps = prep_in_maps(inputs, cfg, cores)
    res = run_bass_kernel_spmd(nc, in_maps, core_ids=list(range(cfg["NC"])),
                               trace=trace)
    LAST_RESULTS = res
    zu = np.concatenate([res.results[c]["zu_out"] for c in range(cfg["NC"])],
                        axis=0)[:cfg["NU"]]
    zi = res.results[0]["zi_out"][:cfg["NI"]]
    return zu.astype(np.float32), zi.astype(np.float32)
